# revision 27
# baseline (speedup 1.0000x reference)
"""GAT 2-layer (HAN) kernel for Trainium2, 8 NeuronCores.

dst-sharding: edges (incl. one self-loop per node) sorted by destination;
each core owns 12544 destination nodes, so segment-softmax and aggregation
are fully local. Between layers, an AllGather of per-node feature rows
[12544, 136] bf16 per core rebuilds the replicated gather table.

Per layer, per core: Haug table [N_pad+1, 136] bf16 = [h | as | ad] rows
(via matmul with host-precomputed W_ext = [W | W@Asel | W@Adsel]).
Edge pass over uniform 128-edge tiles (TW tiles per 32-node window):
  gather Haug[src] (indirect DMA) -> G; ad[dst] via one-hotT matmul;
  ex = exp(lrelu(as_src + ad_dst)) on ACT; G.h *= ex (broadcast);
  scatter psum[32w:32w+32, :] += onehot.T @ [G.h | ex] (col-tiled matmul).
Epilogue per 128-node group: out = num/(den+eps) (+bias); layer 1 also
transposes out rows and matmuls by W2_ext to build layer-2 Haug rows.
"""

import numpy as np

HEADS = 4
OUT = 32
D = HEADS * OUT  # 128
EPS = 1e-16
NEG_SLOPE = 0.2

N = 100000
NCORES = 8
P = 128
WN = 32
WPG = 4
NR = 12544
N_PAD = NR * NCORES  # 100352
GROUPS = NR // P  # 98
HW = 136  # 128 h | 4 as | 4 ad
DUMMY = N_PAD


def _prep_core_tables(src_all, dst_all):
    src = np.concatenate([src_all, np.arange(N, dtype=np.int64)])
    dst = np.concatenate([dst_all, np.arange(N, dtype=np.int64)])
    order = np.argsort(dst, kind="stable")
    src = src[order].astype(np.int32)
    dst = dst[order].astype(np.int32)

    NWIN = N_PAD // WN
    win_of_edge = dst // WN
    counts = np.bincount(win_of_edge, minlength=NWIN)
    TW = max(int(np.max(np.ceil(counts / P))), 1)
    TPG = WPG * TW
    TILES = GROUPS * TPG

    idx = np.full((NCORES, P, TILES), DUMMY, dtype=np.int32)
    oh = np.zeros((NCORES, TILES, P, WN), dtype=np.float32)

    win_starts = np.zeros(NWIN + 1, dtype=np.int64)
    np.cumsum(counts, out=win_starts[1:])

    wpc = NR // WN  # windows per core
    for c in range(NCORES):
        for gw_local in range(wpc):
            gw = c * wpc + gw_local
            s, e = int(win_starts[gw]), int(win_starts[gw + 1])
            cnt = e - s
            wsrc = src[s:e]
            wdstl = dst[s:e] - gw * WN
            for t in range(TW):
                lo = t * P
                if lo >= cnt:
                    break
                hi = min(lo + P, cnt)
                n = hi - lo
                tile = gw_local * TW + t
                idx[c, :n, tile] = wsrc[lo:hi]
                oh[c, tile, np.arange(n), wdstl[lo:hi]] = 1.0

    # pack for contiguous per-group DMA loads:
    # ohp [G, P, TPG*WN], ohTp [G, WN, TPG*P]
    oh4 = oh.reshape(NCORES, GROUPS, TPG, P, WN)
    ohp = np.ascontiguousarray(oh4.transpose(0, 1, 3, 2, 4)).reshape(
        NCORES, GROUPS, P, TPG * WN
    )
    ohTp = np.ascontiguousarray(oh4.transpose(0, 1, 4, 2, 3)).reshape(
        NCORES, GROUPS, WN, TPG * P
    )
    return TW, TILES, idx, ohp, ohTp


def _sel_block(a):
    s = np.zeros((D, HEADS), dtype=np.float64)
    for h in range(HEADS):
        s[h * OUT : (h + 1) * OUT, h] = np.asarray(a, dtype=np.float64)[h]
    return s


_CACHE = {}


def _get_compiled(TW, has_b1, has_b2):
    key = (TW, has_b1, has_b2)
    if key in _CACHE:
        return _CACHE[key]

    import concourse.bass as bass
    import concourse.tile as tile
    from concourse import bacc, mybir
    from concourse.masks import make_identity

    bf16 = mybir.dt.bfloat16
    f32 = mybir.dt.float32
    i32 = mybir.dt.int32
    AF = mybir.ActivationFunctionType
    ALU = mybir.AluOpType

    TPG = WPG * TW
    TILES = GROUPS * TPG

    nc = bacc.Bacc("TRN2", target_bir_lowering=False, debug=False, num_devices=NCORES)

    xT_in = nc.dram_tensor("xT", [P, NR], bf16, kind="ExternalInput")
    w1e_in = nc.dram_tensor("w1e", [P, HW], bf16, kind="ExternalInput")
    w2e_in = nc.dram_tensor("w2e", [P, HW], bf16, kind="ExternalInput")
    idx_in = nc.dram_tensor("idx", [P, TILES], i32, kind="ExternalInput")
    oh_in = nc.dram_tensor("oh", [GROUPS, P, TPG * WN], bf16, kind="ExternalInput")
    ohT_in = nc.dram_tensor("ohT", [GROUPS, WN, TPG * P], bf16, kind="ExternalInput")
    b1_in = nc.dram_tensor("b1bc", [P, D], bf16, kind="ExternalInput") if has_b1 else None
    b2_in = nc.dram_tensor("b2bc", [P, D], f32, kind="ExternalInput") if has_b2 else None
    out_t = nc.dram_tensor("y", [NR, D], f32, kind="ExternalOutput")

    with tile.TileContext(nc) as tc:
        with (
            tc.tile_pool(name="const", bufs=1) as cpool,
            tc.tile_pool(name="sb", bufs=3) as sb,
            tc.tile_pool(name="ep", bufs=2) as ep,
            tc.tile_pool(name="psA", bufs=2, space="PSUM") as ppA,
            tc.tile_pool(name="psE", bufs=2, space="PSUM") as ppE,
            tc.tile_pool(name="psO", bufs=2, space="PSUM") as ppO,
            tc.tile_pool(name="dram", bufs=1, space="DRAM") as dram,
        ):
            xT = cpool.tile([P, NR], bf16)
            nc.sync.dma_start(out=xT[:], in_=xT_in[:])
            w1e = cpool.tile([P, HW], bf16)
            nc.sync.dma_start(out=w1e[:], in_=w1e_in[:])
            w2e = cpool.tile([P, HW], bf16)
            nc.sync.dma_start(out=w2e[:], in_=w2e_in[:])
            idxs = cpool.tile([P, TILES], i32)
            nc.sync.dma_start(out=idxs[:], in_=idx_in[:])
            ident = cpool.tile([P, P], bf16)
            make_identity(nc, ident[:])
            if has_b1:
                b1bc = cpool.tile([P, D], bf16)
                nc.sync.dma_start(out=b1bc[:], in_=b1_in[:])
            if has_b2:
                b2bc = cpool.tile([P, D], f32)
                nc.sync.dma_start(out=b2bc[:], in_=b2_in[:])

            haug1 = dram.tile([N_PAD + 1, HW], bf16)
            haug2 = dram.tile([N_PAD + 1, HW], bf16)
            bounce1 = dram.tile([NR, HW], bf16)
            bounce2 = dram.tile([NR, HW], bf16)

            adw1 = cpool.tile([WN, GROUPS * WPG * HEADS], bf16)
            adw2 = cpool.tile([WN, GROUPS * WPG * HEADS], bf16)

            zrow = cpool.tile([1, HW], bf16)
            nc.vector.memset(zrow[:], 0.0)

            # ---- phase 0: layer-1 Haug slice ----
            for g in range(GROUPS):
                ph = ppA.tile([P, HW], f32, tag="ph", space="PSUM")
                nc.tensor.matmul(
                    out=ph[:], lhsT=xT[:, g * P : (g + 1) * P], rhs=w1e[:],
                    start=True, stop=True,
                )
                hrow = sb.tile([P, HW], bf16, tag="hrow")
                nc.scalar.copy(out=hrow[:], in_=ph[:])
                nc.sync.dma_start(out=bounce1[g * P : (g + 1) * P, :], in_=hrow[:])
                for w in range(WPG):
                    nc.sync.dma_start(
                        out=adw1[:, (g * WPG + w) * HEADS : (g * WPG + w + 1) * HEADS],
                        in_=hrow[w * WN : (w + 1) * WN, D + HEADS : D + 2 * HEADS],
                    )
            nc.gpsimd.collective_compute(
                "AllGather", mybir.AluOpType.bypass,
                replica_groups=[list(range(NCORES))],
                ins=[bounce1[:].opt()],
                outs=[haug1[0:N_PAD, :].opt()],
            )
            nc.sync.dma_start(out=haug1[N_PAD : N_PAD + 1, :], in_=zrow[:])

            def edge_pass(layer, haug, adw):
                last = layer == 2
                for g in range(GROUPS):
                    gbase = g * TPG
                    G = sb.tile([P, TPG * HW], bf16, tag="G")
                    Gv = G[:].rearrange("p (t x) -> p t x", x=HW)
                    pe = ppE.tile([P, TPG * HEADS], f32, tag="pe", space="PSUM")
                    po = ppO.tile([P, D + HEADS], f32, tag="po", space="PSUM")
                    ohg = sb.tile([P, TPG * WN], bf16, tag="ohg")
                    ohTg = sb.tile([WN, TPG * P], bf16, tag="ohTg")
                    nc.sync.dma_start(out=ohg[:], in_=oh_in[g])
                    nc.sync.dma_start(out=ohTg[:], in_=ohT_in[g])
                    for t in range(TPG):
                        nc.gpsimd.indirect_dma_start(
                            out=G[:, t * HW : (t + 1) * HW],
                            out_offset=None,
                            in_=haug[:],
                            in_offset=bass.IndirectOffsetOnAxis(
                                ap=idxs[:, gbase + t : gbase + t + 1], axis=0
                            ),
                        )
                        w = t // TW
                        nc.tensor.matmul(
                            out=pe[:, t * HEADS : (t + 1) * HEADS],
                            lhsT=ohTg[:, t * P : (t + 1) * P],
                            rhs=adw[:, (g * WPG + w) * HEADS : (g * WPG + w + 1) * HEADS],
                            start=True, stop=True,
                        )
                    esb = sb.tile([P, TPG * HEADS], f32, tag="esb")
                    nc.vector.tensor_tensor(
                        out=esb[:],
                        in0=Gv[:, :, D : D + HEADS],
                        in1=pe[:].rearrange("p (t h) -> p t h", h=HEADS),
                        op=ALU.add,
                    )
                    ex1 = sb.tile([P, TPG * HEADS], f32, tag="ex1")
                    ex2 = sb.tile([P, TPG * HEADS], f32, tag="ex2")
                    nc.scalar.activation(out=ex1[:], in_=esb[:], func=AF.Exp)
                    nc.scalar.activation(
                        out=ex2[:], in_=esb[:], func=AF.Exp, scale=NEG_SLOPE
                    )
                    nc.vector.tensor_tensor(
                        out=Gv[:, :, D : D + HEADS],
                        in0=ex1[:].rearrange("p (t h) -> p t h", h=HEADS),
                        in1=ex2[:].rearrange("p (t h) -> p t h", h=HEADS),
                        op=ALU.max,
                    )
                    nc.vector.tensor_tensor(
                        out=Gv[:, :, 0:D].rearrange("p t (h f) -> p t h f", f=OUT),
                        in0=Gv[:, :, 0:D].rearrange("p t (h f) -> p t h f", f=OUT),
                        in1=Gv[:, :, D : D + HEADS]
                        .unsqueeze(-1)
                        .to_broadcast([P, TPG, HEADS, OUT]),
                        op=ALU.mult,
                    )
                    for t in range(TPG):
                        w = t // TW
                        s = t % TW
                        nc.tensor.matmul(
                            out=po[w * WN : (w + 1) * WN, :],
                            lhsT=ohg[:, t * WN : (t + 1) * WN],
                            rhs=G[:, t * HW : t * HW + D + HEADS],
                            start=(s == 0), stop=(s == TW - 1),
                            tile_position=(0, w * WN),
                        )
                    den = ep.tile([P, HEADS], f32, tag="den")
                    nc.vector.tensor_scalar(
                        out=den[:], in0=po[:, D : D + HEADS], scalar1=EPS,
                        scalar2=None, op0=ALU.add,
                    )
                    rec = ep.tile([P, HEADS], f32, tag="rec")
                    nc.vector.reciprocal(out=rec[:], in_=den[:])
                    recb = rec[:].unsqueeze(-1).to_broadcast([P, HEADS, OUT])
                    if last:
                        yrow = ep.tile([P, D], f32, tag="yrow")
                        nc.vector.tensor_tensor(
                            out=yrow[:].rearrange("p (h f) -> p h f", f=OUT),
                            in0=po[:, 0:D].rearrange("p (h f) -> p h f", f=OUT),
                            in1=recb, op=ALU.mult,
                        )
                        if has_b2:
                            nc.vector.tensor_tensor(
                                out=yrow[:], in0=yrow[:], in1=b2bc[:], op=ALU.add
                            )
                        nc.sync.dma_start(out=out_t[g * P : (g + 1) * P, :], in_=yrow[:])
                    else:
                        o1 = ep.tile([P, D], bf16, tag="o1")
                        nc.vector.tensor_tensor(
                            out=o1[:].rearrange("p (h f) -> p h f", f=OUT),
                            in0=po[:, 0:D].rearrange("p (h f) -> p h f", f=OUT),
                            in1=recb, op=ALU.mult,
                        )
                        if has_b1:
                            nc.vector.tensor_tensor(
                                out=o1[:], in0=o1[:], in1=b1bc[:], op=ALU.add
                            )
                        pt = ppA.tile([P, P], bf16, tag="pt", space="PSUM")
                        nc.tensor.transpose(out=pt[:], in_=o1[:], identity=ident[:])
                        o1T = ep.tile([P, P], bf16, tag="o1T")
                        nc.scalar.copy(out=o1T[:], in_=pt[:])
                        ph2 = ppA.tile([P, HW], f32, tag="ph", space="PSUM")
                        nc.tensor.matmul(
                            out=ph2[:], lhsT=o1T[:], rhs=w2e[:], start=True, stop=True
                        )
                        hrow2 = sb.tile([P, HW], bf16, tag="hrow")
                        nc.scalar.copy(out=hrow2[:], in_=ph2[:])
                        nc.sync.dma_start(
                            out=bounce2[g * P : (g + 1) * P, :], in_=hrow2[:]
                        )
                        for w in range(WPG):
                            nc.sync.dma_start(
                                out=adw2[
                                    :, (g * WPG + w) * HEADS : (g * WPG + w + 1) * HEADS
                                ],
                                in_=hrow2[
                                    w * WN : (w + 1) * WN, D + HEADS : D + 2 * HEADS
                                ],
                            )

            edge_pass(1, haug1, adw1)
            nc.gpsimd.collective_compute(
                "AllGather", mybir.AluOpType.bypass,
                replica_groups=[list(range(NCORES))],
                ins=[bounce2[:].opt()],
                outs=[haug2[0:N_PAD, :].opt()],
            )
            nc.sync.dma_start(out=haug2[N_PAD : N_PAD + 1, :], in_=zrow[:])
            edge_pass(2, haug2, adw2)

    nc.compile()
    _CACHE[key] = nc
    return nc


def _prepare(x, edge_index, W1, a_src1, a_dst1, b1, W2, a_src2, a_dst2, b2):
    import ml_dtypes

    x = np.asarray(x, dtype=np.float32)
    ei = np.asarray(edge_index)

    TW, TILES, idx, ohp, ohTp = _prep_core_tables(
        ei[0].astype(np.int64), ei[1].astype(np.int64)
    )

    def ext(W, a_s, a_d):
        W = np.asarray(W, dtype=np.float64)
        return np.concatenate([W, W @ _sel_block(a_s), W @ _sel_block(a_d)], axis=1)

    w1e = ext(W1, a_src1, a_dst1).astype(ml_dtypes.bfloat16)
    w2e = ext(W2, a_src2, a_dst2).astype(ml_dtypes.bfloat16)

    has_b1 = bool(np.any(np.asarray(b1)))
    has_b2 = bool(np.any(np.asarray(b2)))

    nc = _get_compiled(TW, has_b1, has_b2)

    xpad = np.zeros((N_PAD, D), dtype=np.float32)
    xpad[:N] = x
    in_maps = []
    for c in range(NCORES):
        m = {
            "xT": np.ascontiguousarray(xpad[c * NR : (c + 1) * NR].T).astype(
                ml_dtypes.bfloat16
            ),
            "w1e": w1e,
            "w2e": w2e,
            "idx": np.ascontiguousarray(idx[c]),
            "oh": ohp[c].astype(ml_dtypes.bfloat16),
            "ohT": ohTp[c].astype(ml_dtypes.bfloat16),
        }
        if has_b1:
            m["b1bc"] = np.tile(np.asarray(b1, np.float32)[None, :], (P, 1)).astype(
                ml_dtypes.bfloat16
            )
        if has_b2:
            m["b2bc"] = np.tile(np.asarray(b2, np.float32)[None, :], (P, 1))
        in_maps.append(m)
    return nc, in_maps


def kernel(x, edge_index, W1, a_src1, a_dst1, b1, W2, a_src2, a_dst2, b2):
    from concourse import bass_utils

    nc, in_maps = _prepare(
        x, edge_index, W1, a_src1, a_dst1, b1, W2, a_src2, a_dst2, b2
    )
    res = bass_utils.run_bass_kernel_spmd(
        nc, in_maps, core_ids=list(range(NCORES)), trace=False
    )
    outs = [r["y"] for r in res.results]
    return np.concatenate(outs, axis=0)[:N].astype(np.float32)

